# revision 41
# baseline (speedup 1.0000x reference)
"""NT-Xent loss kernel for Trainium2 (8 NeuronCores, SPMD row-sharded).

Reference computation (N=4096, D=256, T=0.5):
    zi, zj = l2norm(z_i), l2norm(z_j); reps = concat([zi, zj])  # [2N, D]
    sim = reps @ reps.T
    lse_a = logsumexp over row a of sim/T with the diagonal excluded
    pos_a = sim[a, a+-N]
    loss = mean(lse_a - pos_a/T)

v4 design (symmetric chunk-cover, on top of the v3 fp8 pipeline):
  * Row->(tile, partition) map sigma(g, p) = (g//8)*1024 + p*8 + (g%8):
    every HBM load descriptor is one contiguous 8KB run per partition.
    Core c owns global tiles {c + 8k} -- exactly one tile per 1024-column
    chunk k, so a chunk-granular symmetric cover maps cleanly onto cores.
  * exp(2*sim) is needed for every ordered pair, but the matrix is
    symmetric: each unordered CHUNK pair is computed once.  For column
    chunk C the cores compute row tiles k in {C, C-1, C-2, C-3} (mod 8)
    plus {C-4} when C >= 4: 36 of 64 chunk-blocks per core.  ACT exp
    work drops ~45%.
  * Row sums come from the ACT accumulator (as in v3).  Column sums of
    each exp block (the transposed pair's contribution) come from
    ones-vector matmuls on the otherwise idle PE, accumulated in a
    [16, 512] PSUM tile (partition row = (chunk, half)) and shipped to
    the host, which adds them into the right global rows.
  * Normalize-to-fp8 runs on DVE for even chunks and on the GpSimd/Pool
    engine for odd chunks so DVE preprocessing stays off the critical
    path.
  * Main loop is column-outer; chunk pipelines (sumsq -> invn ->
    normalize -> XBAR pair-transpose) are emitted two chunks ahead.
"""

import sys

for _p in ("/opt/trn_rl_repo",):
    if _p not in sys.path:
        sys.path.insert(0, _p)

import numpy as np
from contextlib import ExitStack

import concourse.bass as bass
import concourse.tile as tile
from concourse import mybir
from concourse.masks import make_identity
from concourse.vector_clock import ScopedClock as _ScopedClock


def _patched_drain_and_barrier(self, tick_clock, wait_clock):
    """Tile's closing drain carries one sem-wait per DMA lane used, but this
    walrus build only accepts a single sync wait on a Drain (CTRL-NO)
    lowering ("Too many sync wait commands").  Split the waits across a
    chain of drains (sequential on SP, so semantics are unchanged)."""
    nc = self.nc
    drain_inst = nc.sync.drain()
    wait_clock.add_sem_waits(
        drain_inst.ins, _ScopedClock({None: tick_clock.global_clock})
    )
    si = drain_inst.ins.sync_info
    if si is not None:
        waits = list(si.on_wait or [])
        if len(waits) > 1:
            import bass_rust as _br

            si.on_wait = waits[:1]
            for w in waits[1:]:
                d2 = nc.sync.drain()
                d2.ins.sync_info = _br.SyncInfo(on_wait=[w], on_update=[])
    nc.all_engine_barrier()
    assert self.sems is not None
    popped = nc._tile_sem_poison_stack.pop()
    assert popped is self._sem_poison
    nc.clear_and_free_semaphores(list(self.sems.allocated().values()))
    nc.all_engine_barrier()


tile.TileContext._drain_and_barrier = _patched_drain_and_barrier

_orig_lower_ordered = tile.TileContext._lower_ordered_insts


def _split_multiwaits_and_lower(self, ordered):
    """Same walrus limitation as above, for scheduled compute/DMA
    instructions: hoist all but one sync wait onto single-wait NoOps that
    precede the instruction on its own engine."""
    nc = self.nc
    for insts in ordered.values():
        if not any(
            inst.sync_info is not None and len(inst.sync_info.on_wait or []) > 1
            for inst in insts
        ):
            continue
        out = []
        for inst in insts:
            si = inst.sync_info
            waits = list(si.on_wait) if si is not None and si.on_wait else []
            if len(waits) > 1 and getattr(inst, "engine", None) is not None:
                for w in waits[:-1]:
                    out.append(
                        mybir.InstNoOp(
                            name=nc.get_next_instruction_name(),
                            sync_info=mybir.SyncInfo(on_wait=[w], on_update=[]),
                            bass_nofuse=True,
                            engine=inst.engine,
                        )
                    )
                si.on_wait = waits[-1:]
            out.append(inst)
        insts[:] = out
    return _orig_lower_ordered(self, ordered)


tile.TileContext._lower_ordered_insts = _split_multiwaits_and_lower

N_CORES = 8
N_FULL = 4096
D_FULL = 256

f32 = mybir.dt.float32
bf16 = mybir.dt.bfloat16
fp8 = mybir.dt.float8e4
ALU = mybir.AluOpType
AF = mybir.ActivationFunctionType
AX = mybir.AxisListType


def build_bass(N=N_FULL, D=D_FULL, n_cores=N_CORES):
    n2 = 2 * N
    R = n2 // n_cores          # rows per core (1024)
    TF = n2 // 128             # full 128-row tiles (64)
    TB = R // 128              # per-core row tiles (8) == chunks (NCH)
    CH = 8                     # tiles per chunk (1024 cols)
    NCH = TF // CH             # chunks (8)
    RED1 = 16                  # stage-1 reduce group size
    TH = TB // 2
    NSLOT = 5                  # max chunk-blocks per row tile (k < 4)

    assert R % 128 == 0 and D == 256 and TB == NCH == 8

    nc = bass.Bass()
    z_i = nc.declare_dram_parameter("z_i", [N, D], f32, isOutput=False)
    z_j = nc.declare_dram_parameter("z_j", [N, D], f32, isOutput=False)
    zb = nc.declare_dram_parameter("zb", [R, D], f32, isOutput=False)
    lse_out = nc.declare_dram_parameter("lse_in", [128, TB], f32, isOutput=True)
    pos_out = nc.declare_dram_parameter("posd", [128, TH], f32, isOutput=True)
    cs_out = nc.declare_dram_parameter("colsum", [2 * NCH, 512], f32, isOutput=True)

    with ExitStack() as ctx:
        tc = ctx.enter_context(tile.TileContext(nc))
        big = ctx.enter_context(tc.tile_pool(name="big", bufs=1))
        f8p = ctx.enter_context(tc.tile_pool(name="f8p", bufs=3))
        sqp = ctx.enter_context(tc.tile_pool(name="sqp", bufs=2))
        epool = ctx.enter_context(tc.tile_pool(name="epool", bufs=10))
        pmm = ctx.enter_context(tc.tile_pool(name="pmm", bufs=3, space="PSUM"))
        pcs = ctx.enter_context(tc.tile_pool(name="pcs", bufs=1, space="PSUM"))

        zf = big.tile([128, TF, D], bf16)    # all reps rows, bf16 raw
        zbn = big.tile([128, TB, D], bf16)   # this core's rows, bf16 raw
        ssq = big.tile([128, TF + TB], f32)
        lnssq = big.tile([128, TF + TB], f32)
        invn = big.tile([128, TF + TB], f32)
        # Transposed normalized fp8, stored as bf16-typed fake pairs:
        # repsT[p, t, r] (bf16) == features (2p, 2p+1) of row sigma(t, r),
        # packed as two consecutive fp8 bytes.
        repsT = big.tile([128, TF, 128], bf16)
        ident = big.tile([128, 128], bf16)
        make_identity(nc, ident)
        ones_b = big.tile([128, 1], bf16)
        NCS = 2 * NCH
        selq = big.tile([128, NCS, NCS], fp8)
        selp = big.tile([128, NCS, 2, NCS], fp8)  # DoubleRow pair selector
        inv2 = big.tile([128, TB], f32)           # 2 * invn of own rows
        Spart = big.tile([128, TB, NSLOT], f32)
        # k >= 4 row tiles only get 4 chunk-blocks; zero their 5th slot.
        nc.scalar.memzero(Spart[:, 4:TB, NSLOT - 1 : NSLOT])
        cs_sb = big.tile([NCS, 512], f32)   # colsum staging, row = (C, h)
        csacc = pcs.tile([NCS, 512], f32)   # colsum PSUM accumulator

        def chunk_src(c):
            rows = c * (CH * 128)
            za, off = (z_i, rows) if rows < N else (z_j, rows - N)
            return za[off : off + CH * 128, :].rearrange("(p k) d -> p k d", k=CH)

        zb_r = zb[:, :].rearrange("(p k) d -> p k d", k=TB)

        def two_stage_sumsq(src, ntiles, qsl, tag):
            sq = sqp.tile([128, ntiles * D // RED1, RED1], bf16, tag=tag)
            s1 = sqp.tile([128, ntiles, D // RED1], bf16, tag=tag + "1")
            src3 = src.rearrange("p t (g r) -> p (t g) r", r=RED1)
            nc.vector.tensor_tensor(out=sq, in0=src3, in1=src3, op=ALU.mult)
            with nc.allow_low_precision("bf16 stage-1 partial sums of 16"):
                nc.vector.tensor_reduce(
                    out=s1.rearrange("p t g -> p (t g)"), in_=sq, op=ALU.add,
                    axis=AX.X,
                )
            nc.vector.reduce_sum(out=ssq[:, qsl], in_=s1, axis=AX.X)

        sqscr = big.tile([128, D], f32)  # ACT Square scratch output

        def chunk_dve(dst, t0, ntiles, ssq0, act_sumsq=False):
            sl = slice(t0, t0 + ntiles)
            qsl = slice(ssq0, ssq0 + ntiles)
            if act_sumsq:
                # Fill-phase sumsq on the (idle) ACT engine: Square with
                # f32 accumulator, one instruction per 128x256 tile.
                for j in range(ntiles):
                    nc.scalar.activation(
                        out=sqscr, in_=dst[:, t0 + j, :], func=AF.Square,
                        accum_out=ssq[:, ssq0 + j : ssq0 + j + 1],
                    )
            else:
                two_stage_sumsq(dst[:, sl, :], ntiles, qsl, "sq")
            nc.scalar.activation(out=lnssq[:, qsl], in_=ssq[:, qsl], func=AF.Ln)
            nc.scalar.activation(
                out=invn[:, qsl], in_=lnssq[:, qsl], func=AF.Exp, scale=-0.5
            )
            z8 = f8p.tile([128, ntiles, D], fp8, tag="z8")
            for j in range(ntiles):
                nc.vector.tensor_scalar_mul(
                    out=z8[:, j, :], in0=dst[:, t0 + j, :],
                    scalar1=invn[:, ssq0 + j : ssq0 + j + 1],
                )
            return z8

        def chunk_pipeline(cc):
            """sumsq -> invn -> normalize-to-fp8 -> XBAR pair-transpose:
            repsT[p, 8cc+t, r] = pair (2p, 2p+1) of (tile 8cc+t, row r).
            Two half-chunk transposes so the first sim strip of the
            consuming step unlocks before the whole chunk lands."""
            z8 = chunk_dve(zf, cc * CH, CH, cc * CH)
            for hh in range(2):
                nc.sync.dma_start_transpose(
                    out=repsT[:, cc * CH + 4 * hh : cc * CH + 4 * (hh + 1), :],
                    in_=z8[:, 4 * hh : 4 * (hh + 1), :].bitcast(bf16),
                )

        def pe_transpose(z8, ntiles):
            """PE transpose of the fake-bf16 pair tiles via matmul x
            identity (bf16 -> f32 PSUM is exact); same pair layout as the
            XBAR after a cast-copy.  Used only before the main loop."""
            tps = pmm.tile([128, 1024], f32, tag="ps")
            z8b = z8[:, :, :].bitcast(bf16)
            for t in range(ntiles):
                nc.tensor.matmul(
                    out=tps[:, t * 128 : (t + 1) * 128],
                    lhsT=z8b[:, t, :], rhs=ident,
                    start=True, stop=True,
                )
            return tps

        def pair_ap(ap_bf16):
            """[128, t, 128] fake-bf16 -> [128(K), 2(slot), t*128] fp8 AP
            for DoubleRow matmul operands."""
            return ap_bf16.bitcast(fp8).rearrange("p t (r b) -> p b (t r)", b=2)

        # ---- first loads; later loads are emitted interleaved with the
        # chunk pipelines so each XBAR transpose's conservative cross-DMA
        # ordering dep lands on an already-completed load ----
        nc.gpsimd.dma_start(out=zbn[:, :, :], in_=zb_r)
        for cc in range(3):
            nc.gpsimd.dma_start(out=zf[:, cc * CH : (cc + 1) * CH, :], in_=chunk_src(cc))

        # ---- per-core row block first: it gates every main-loop matmul.
        # The stationary is RAW fp8 (no normalization): the row norm is
        # folded into the exp as a per-partition vector scale 2*invn_a,
        # so the PE-transpose chain starts right after the zb load instead
        # of after zb's sumsq.  PE transpose + direct PSUM repack into the
        # slot-major contiguous stationary (LDWEIGHTS dual-fp8 rejects
        # stride-2 operands). ----
        znbT8 = big.tile([128, 2, R], fp8)
        z8zb = chunk_dve(zbn, 0, TB, TF, act_sumsq=True)
        tzb = pe_transpose(z8zb, TB)

        # ---- chunks 0/1 bypass the XBAR (PE transpose + DVE cast-copy)
        # so the XBAR queue starts on chunk 2 and never gates the fill;
        # c1's PE work is deferred into main step 0 (it only gates step 1).
        # Chunk 0's pipeline runs before the znbT8 repack on DVE: both
        # gate the first sim matmul, but c0's chain is longer.
        def pe_chunk_finish(cc):
            tpsc = pe_transpose(_z8c[cc], CH)
            nc.scalar.copy(
                out=repsT[:, cc * CH : (cc + 1) * CH, :],
                in_=tpsc[:, 0 : CH * 128],
            )

        _z8c = {}
        _z8c[0] = chunk_dve(zf, 0, CH, 0, act_sumsq=True)
        pe_chunk_finish(0)
        nc.scalar.copy(
            out=znbT8,
            in_=tzb.bitcast(fp8).rearrange("p (x q) -> p q x", q=4)[
                :, 2:4, 0 : TB * 128
            ],
        )

        # Selector stationaries for the colsum matmuls: selq[:, m, j] is 1
        # only at j == m, so ones^T @ E lands in partition row m of the
        # long-lived [16, 512] PSUM accumulator (matmul output base
        # partition must be 0) while the other rows accumulate zeros.
        # Built here so the tiny DVE ops stay off the fill critical path.
        with nc.allow_low_precision("exact: rows of identity sum to 1.0"):
            nc.vector.reduce_sum(out=ones_b, in_=ident, axis=AX.X)
        nc.scalar.memzero(selq)
        nc.scalar.memzero(selp)
        for m in range(NCS):
            nc.vector.tensor_copy(out=selq[:, m, m : m + 1], in_=ones_b)
            for s in range(2):
                nc.vector.tensor_copy(out=selp[:, m, s, m : m + 1], in_=ones_b)

        # ---- main loop: column-outer over chunks; symmetric cover.
        # Emission order per step: sim matmuls + exps for C, with the
        # colsum matmuls of step C-1 interleaved between blocks (their E
        # tiles are long done, so they keep the PE streaming instead of
        # stalling on this step's exps), then chunk C+2's preprocessing
        # pipeline (so its LN/EXP sit behind this step's exps in the ACT
        # queue).  Pairs of E tiles are byte-interleaved so one DoubleRow
        # ones-matmul column-sums both at once. ----
        cs_first = True
        pend = []  # deferred colsum matmul thunks from the previous chunk

        def colsum_mm(Cp, h, entry, stop):
            nonlocal cs_first
            kind, e = entry
            m = 2 * Cp + h
            if kind == "pair":
                nc.tensor.matmul(
                    out=csacc[:, :],
                    lhsT=selp[:, m, :, :],
                    rhs=e[:, h * 512 : (h + 1) * 512, :].rearrange(
                        "p x b -> p b x"
                    ),
                    start=cs_first, stop=stop,
                    perf_mode=mybir.MatmulPerfMode.DoubleRow,
                    skip_group_check=True,
                )
            else:
                nc.tensor.matmul(
                    out=csacc[:, :],
                    lhsT=selq[:, m, :],
                    rhs=e[:, h * 512 : (h + 1) * 512],
                    start=cs_first, stop=stop,
                    skip_group_check=True,
                )
            cs_first = False

        for C in range(NCH):
            if C + 3 < NCH:
                nc.gpsimd.dma_start(
                    out=zf[:, (C + 3) * CH : (C + 4) * CH, :], in_=chunk_src(C + 3)
                )
            ks = [(C - d) % NCH for d in range(4)]
            if C >= 4:
                ks.append(C - 4)
            cj = [k for k in ks if k != C]
            # pair up the colsum jobs: (cj0,cj1) share one byte-interleaved
            # tile, (cj2,cj3) another; a leftover third job stays single.
            pair_of = {}
            for i in range(0, len(cj) - 1, 2):
                ep = epool.tile([128, 1024, 2], fp8, tag="ep")
                pair_of[cj[i]] = (ep, 0)
                pair_of[cj[i + 1]] = (ep, 1)
            jobs = []
            for bi, k in enumerate(ks):
                ps = pmm.tile([128, 1024], f32, tag="ps")
                for h in range(2):
                    nc.tensor.matmul(
                        out=ps[:, h * 512 : (h + 1) * 512],
                        lhsT=znbT8[:, :, k * 128 : (k + 1) * 128],
                        rhs=pair_ap(repsT[:, C * CH + 4 * h : C * CH + 4 * h + 4, :]),
                        start=True, stop=True,
                        perf_mode=mybir.MatmulPerfMode.DoubleRow,
                    )
                slot = (C - k) % NCH
                if k == C:
                    e_out = epool.tile([128, 1024], fp8, tag="e")
                    out_ap = e_out
                elif k in pair_of:
                    ep, s = pair_of[k]
                    out_ap = ep[:, :, s]
                    if s == 1:
                        jobs.append(("pair", ep))
                else:
                    e_out = epool.tile([128, 1024], fp8, tag="e")
                    out_ap = e_out
                    jobs.append(("single", e_out))
                # exp(2*sim) in [e^-4, e^4] sits inside fp8e4m3 range; the
                # per-element quantization noise is zero-mean and averages
                # out across the 512-term column sums.  The row norm
                # 2*invn_a rides in as a per-partition vector scale.
                nc.scalar.activation(
                    out=out_ap, in_=ps, func=AF.Exp, scale=2.0,
                    accum_out=Spart[:, k, slot : slot + 1],
                )
                if pend:
                    colsum_mm(*pend.pop(0), stop=False)
            if C == 0:
                _z8c[1] = chunk_dve(zf, CH, CH, CH, act_sumsq=True)
                pe_chunk_finish(1)
            while pend:
                colsum_mm(*pend.pop(0), stop=False)
            pend = [(C, h, entry) for h in range(2) for entry in jobs]
            if C + 2 < NCH:
                chunk_pipeline(C + 2)
        for i, (Cp, h, entry) in enumerate(pend):
            colsum_mm(Cp, h, entry, stop=(i == len(pend) - 1))

        # ---- self-diagonal exp(2*|zn|^2) from ssq * invn^2 (f32, [128,TB]) ----
        qz = slice(TF, TF + TB)
        d1 = big.tile([128, TB], f32)
        dacc = big.tile([128, TB], f32)
        nc.vector.tensor_mul(out=d1, in0=ssq[:, qz], in1=invn[:, qz])
        nc.vector.tensor_mul(out=dacc, in0=d1, in1=invn[:, qz])
        expd = big.tile([128, TB], f32)
        nc.scalar.activation(out=expd, in_=dacc, func=AF.Exp, scale=2.0)

        # ---- positive pairs: raw dots * invn_k * invn_{k+4}, local pairs ----
        posp = sqp.tile([128, TH, D // RED1, RED1], bf16, tag="sq")
        pos1 = sqp.tile([128, TH, D // RED1], bf16, tag="sq1")
        posr = big.tile([128, TH], f32)
        zl = zbn[:, 0:TH, :].rearrange("p t (g r) -> p t g r", r=RED1)
        zh = zbn[:, TH:TB, :].rearrange("p t (g r) -> p t g r", r=RED1)
        nc.vector.tensor_tensor(out=posp, in0=zl, in1=zh, op=ALU.mult)
        with nc.allow_low_precision("bf16 stage-1 partial sums of 16"):
            nc.vector.tensor_reduce(out=pos1, in_=posp, op=ALU.add, axis=AX.X)
        nc.vector.reduce_sum(out=posr, in_=pos1, axis=AX.X)
        ps1 = big.tile([128, TH], f32)
        ps2 = big.tile([128, TH], f32)
        posd = big.tile([128, TH], f32)
        nc.vector.tensor_mul(out=ps1, in0=posr, in1=invn[:, TF : TF + TH])
        nc.vector.tensor_mul(out=ps2, in0=ps1, in1=invn[:, TF + TH : TF + TB])
        nc.vector.tensor_scalar_mul(out=posd, in0=ps2, scalar1=2.0)
        nc.sync.dma_start(out=pos_out[:, :], in_=posd)

        nc.vector.tensor_copy(out=cs_sb, in_=csacc[:, :])
        nc.sync.dma_start(out=cs_out[:, :], in_=cs_sb)

        # ---- S' = sum - diag, ship out ----
        S_t = big.tile([128, TB], f32)
        nc.vector.reduce_sum(out=S_t, in_=Spart[:, :, :], axis=AX.X)
        lse_in_t = big.tile([128, TB], f32)
        nc.vector.tensor_sub(out=lse_in_t, in0=S_t, in1=expd)
        nc.sync.dma_start(out=lse_out[:, :], in_=lse_in_t)

    return nc


_NC_CACHE = {}


def _get_nc(N=N_FULL, D=D_FULL):
    key = (N, D)
    if key not in _NC_CACHE:
        _NC_CACHE[key] = build_bass(N, D)
    return _NC_CACHE[key]


def make_in_maps(z_i, z_j, n_cores=N_CORES):
    z_i = np.ascontiguousarray(z_i, dtype=np.float32)
    z_j = np.ascontiguousarray(z_j, dtype=np.float32)
    reps = np.concatenate([z_i, z_j], axis=0)
    TB = reps.shape[0] // 128 // n_cores
    maps = []
    for c in range(n_cores):
        # core c owns global tiles {c + 8k}; tile g holds rows
        # sigma(g, p) = (g // 8) * 1024 + p * 8 + (g % 8).  zb row
        # (p*TB + k) feeds (partition p, local tile k).
        idx = np.empty(128 * TB, dtype=np.int64)
        for p in range(128):
            for k in range(TB):
                idx[p * TB + k] = k * 1024 + p * 8 + c
        maps.append({"z_i": z_i, "z_j": z_j, "zb": np.ascontiguousarray(reps[idx])})
    return maps


def assemble(results, N=N_FULL, n_cores=N_CORES):
    """Host-side gather + final ln/mean ("all-reduce the mean loss")."""
    n2 = 2 * N
    TB = n2 // 128 // n_cores
    lse = np.empty(n2, dtype=np.float64)
    pos = np.empty(n2, dtype=np.float64)
    colsum = np.zeros(n2, dtype=np.float64)
    p_ar = np.arange(128)
    for c, r in enumerate(results):
        lse_in = np.asarray(r["lse_in"], dtype=np.float64)   # [128, TB]
        posd = np.asarray(r["posd"], dtype=np.float64)       # [128, TB//2]
        for k in range(TB):
            rows = k * 1024 + p_ar * 8 + c   # sigma(c + 8k, p)
            lse[rows] = lse_in[:, k]
            pos[rows] = posd[:, k % (TB // 2)]
        # colsum[2C+h, q]: within-chunk col j = h*512 + q = tc*128 + rr
        # -> global row C*1024 + rr*8 + tc
        cs = np.asarray(r["colsum"], dtype=np.float64).reshape(8, 2, 4, 128)
        colsum += np.transpose(cs, (0, 3, 1, 2)).reshape(n2)
    loss = np.mean(np.log(lse + colsum) - pos)
    return np.float32(loss)


def _run(z_i, z_j, trace=False, tmpdir=None, **spmd_kwargs):
    from concourse.bass_utils import run_bass_kernel_spmd

    N, D = z_i.shape
    nc = _get_nc(N, D)
    in_maps = make_in_maps(z_i, z_j)
    out = run_bass_kernel_spmd(
        nc, in_maps, list(range(N_CORES)), trace=trace, tmpdir=tmpdir, **spmd_kwargs
    )
    return assemble(out.results, N), out


def kernel(z_i, z_j):
    loss, _ = _run(np.asarray(z_i), np.asarray(z_j))
    return loss


if __name__ == "__main__":
    rng = np.random.default_rng(0)
    z_i = rng.standard_normal((N_FULL, D_FULL), dtype=np.float32)
    z_j = rng.standard_normal((N_FULL, D_FULL), dtype=np.float32)
    print(kernel(z_i, z_j))


# revision 42
# speedup vs baseline: 1.0839x; 1.0839x over previous
"""NT-Xent loss kernel for Trainium2 (8 NeuronCores, SPMD row-sharded).

Reference computation (N=4096, D=256, T=0.5):
    zi, zj = l2norm(z_i), l2norm(z_j); reps = concat([zi, zj])  # [2N, D]
    sim = reps @ reps.T
    lse_a = logsumexp over row a of sim/T with the diagonal excluded
    pos_a = sim[a, a+-N]
    loss = mean(lse_a - pos_a/T)

v4 design (symmetric chunk-cover, on top of the v3 fp8 pipeline):
  * Row->(tile, partition) map sigma(g, p) = (g//8)*1024 + p*8 + (g%8):
    every HBM load descriptor is one contiguous 8KB run per partition.
    Core c owns global tiles {c + 8k} -- exactly one tile per 1024-column
    chunk k, so a chunk-granular symmetric cover maps cleanly onto cores.
  * exp(2*sim) is needed for every ordered pair, but the matrix is
    symmetric: each unordered CHUNK pair is computed once.  For column
    chunk C the cores compute row tiles k in {C, C-1, C-2, C-3} (mod 8)
    plus {C-4} when C >= 4: 36 of 64 chunk-blocks per core.  ACT exp
    work drops ~45%.
  * Row sums come from the ACT accumulator (as in v3).  Column sums of
    each exp block (the transposed pair's contribution) come from
    ones-vector matmuls on the otherwise idle PE, accumulated in a
    [16, 512] PSUM tile (partition row = (chunk, half)) and shipped to
    the host, which adds them into the right global rows.
  * Normalize-to-fp8 runs on DVE for even chunks and on the GpSimd/Pool
    engine for odd chunks so DVE preprocessing stays off the critical
    path.
  * Main loop is column-outer; chunk pipelines (sumsq -> invn ->
    normalize -> XBAR pair-transpose) are emitted two chunks ahead.
"""

import sys

for _p in ("/opt/trn_rl_repo",):
    if _p not in sys.path:
        sys.path.insert(0, _p)

import numpy as np
from contextlib import ExitStack

import concourse.bass as bass
import concourse.tile as tile
from concourse import mybir
from concourse.masks import make_identity
from concourse.vector_clock import ScopedClock as _ScopedClock


def _patched_drain_and_barrier(self, tick_clock, wait_clock):
    """Tile's closing drain carries one sem-wait per DMA lane used, but this
    walrus build only accepts a single sync wait on a Drain (CTRL-NO)
    lowering ("Too many sync wait commands").  Split the waits across a
    chain of drains (sequential on SP, so semantics are unchanged)."""
    nc = self.nc
    drain_inst = nc.sync.drain()
    wait_clock.add_sem_waits(
        drain_inst.ins, _ScopedClock({None: tick_clock.global_clock})
    )
    si = drain_inst.ins.sync_info
    if si is not None:
        waits = list(si.on_wait or [])
        if len(waits) > 1:
            import bass_rust as _br

            si.on_wait = waits[:1]
            for w in waits[1:]:
                d2 = nc.sync.drain()
                d2.ins.sync_info = _br.SyncInfo(on_wait=[w], on_update=[])
    nc.all_engine_barrier()
    assert self.sems is not None
    popped = nc._tile_sem_poison_stack.pop()
    assert popped is self._sem_poison
    nc.clear_and_free_semaphores(list(self.sems.allocated().values()))
    nc.all_engine_barrier()


tile.TileContext._drain_and_barrier = _patched_drain_and_barrier

_orig_lower_ordered = tile.TileContext._lower_ordered_insts


def _split_multiwaits_and_lower(self, ordered):
    """Same walrus limitation as above, for scheduled compute/DMA
    instructions: hoist all but one sync wait onto single-wait NoOps that
    precede the instruction on its own engine."""
    nc = self.nc
    for insts in ordered.values():
        if not any(
            inst.sync_info is not None and len(inst.sync_info.on_wait or []) > 1
            for inst in insts
        ):
            continue
        out = []
        for inst in insts:
            si = inst.sync_info
            waits = list(si.on_wait) if si is not None and si.on_wait else []
            if len(waits) > 1 and getattr(inst, "engine", None) is not None:
                for w in waits[:-1]:
                    out.append(
                        mybir.InstNoOp(
                            name=nc.get_next_instruction_name(),
                            sync_info=mybir.SyncInfo(on_wait=[w], on_update=[]),
                            bass_nofuse=True,
                            engine=inst.engine,
                        )
                    )
                si.on_wait = waits[-1:]
            out.append(inst)
        insts[:] = out
    return _orig_lower_ordered(self, ordered)


tile.TileContext._lower_ordered_insts = _split_multiwaits_and_lower

N_CORES = 8
N_FULL = 4096
D_FULL = 256

f32 = mybir.dt.float32
bf16 = mybir.dt.bfloat16
fp8 = mybir.dt.float8e4
ALU = mybir.AluOpType
AF = mybir.ActivationFunctionType
AX = mybir.AxisListType


def build_bass(N=N_FULL, D=D_FULL, n_cores=N_CORES):
    n2 = 2 * N
    R = n2 // n_cores          # rows per core (1024)
    TF = n2 // 128             # full 128-row tiles (64)
    TB = R // 128              # per-core row tiles (8) == chunks (NCH)
    CH = 8                     # tiles per chunk (1024 cols)
    NCH = TF // CH             # chunks (8)
    RED1 = 16                  # stage-1 reduce group size
    TH = TB // 2
    NSLOT = 5                  # max chunk-blocks per row tile (k < 4)

    assert R % 128 == 0 and D == 256 and TB == NCH == 8

    nc = bass.Bass()
    z_i = nc.declare_dram_parameter("z_i", [N, D], f32, isOutput=False)
    z_j = nc.declare_dram_parameter("z_j", [N, D], f32, isOutput=False)
    zb = nc.declare_dram_parameter("zb", [R, D], f32, isOutput=False)
    lse_out = nc.declare_dram_parameter("lse_in", [128, TB], f32, isOutput=True)
    pos_out = nc.declare_dram_parameter("posd", [128, TH], f32, isOutput=True)
    cs_out = nc.declare_dram_parameter("colsum", [2 * NCH, 512], f32, isOutput=True)

    with ExitStack() as ctx:
        tc = ctx.enter_context(tile.TileContext(nc))
        big = ctx.enter_context(tc.tile_pool(name="big", bufs=1))
        f8p = ctx.enter_context(tc.tile_pool(name="f8p", bufs=3))
        sqp = ctx.enter_context(tc.tile_pool(name="sqp", bufs=2))
        epool = ctx.enter_context(tc.tile_pool(name="epool", bufs=10))
        pmm = ctx.enter_context(tc.tile_pool(name="pmm", bufs=3, space="PSUM"))
        pcs = ctx.enter_context(tc.tile_pool(name="pcs", bufs=1, space="PSUM"))

        zf = big.tile([128, TF, D], bf16)    # all reps rows, bf16 raw
        zbn = big.tile([128, TB, D], bf16)   # this core's rows, bf16 raw
        ssq = big.tile([128, TF + TB], f32)
        lnssq = big.tile([128, TF + TB], f32)
        invn = big.tile([128, TF + TB], f32)
        # Transposed normalized fp8, stored as bf16-typed fake pairs:
        # repsT[p, t, r] (bf16) == features (2p, 2p+1) of row sigma(t, r),
        # packed as two consecutive fp8 bytes.
        repsT = big.tile([128, TF, 128], bf16)
        ident = big.tile([128, 128], bf16)
        make_identity(nc, ident)
        ones_b = big.tile([128, 1], bf16)
        NCS = 2 * NCH
        selq = big.tile([128, NCS, NCS], fp8)
        selp = big.tile([128, NCS, 2, NCS], fp8)  # DoubleRow pair selector
        inv2 = big.tile([128, TB], f32)           # 2 * invn of own rows
        Spart = big.tile([128, TB, NSLOT], f32)
        # k >= 4 row tiles only get 4 chunk-blocks; zero their 5th slot.
        nc.scalar.memzero(Spart[:, 4:TB, NSLOT - 1 : NSLOT])
        cs_sb = big.tile([NCS, 512], f32)   # colsum staging, row = (C, h)
        csacc = pcs.tile([NCS, 512], f32)   # colsum PSUM accumulator

        def chunk_src(c):
            rows = c * (CH * 128)
            za, off = (z_i, rows) if rows < N else (z_j, rows - N)
            return za[off : off + CH * 128, :].rearrange("(p k) d -> p k d", k=CH)

        zb_r = zb[:, :].rearrange("(p k) d -> p k d", k=TB)

        def two_stage_sumsq(src, ntiles, qsl, tag):
            sq = sqp.tile([128, ntiles * D // RED1, RED1], bf16, tag=tag)
            s1 = sqp.tile([128, ntiles, D // RED1], bf16, tag=tag + "1")
            src3 = src.rearrange("p t (g r) -> p (t g) r", r=RED1)
            nc.vector.tensor_tensor(out=sq, in0=src3, in1=src3, op=ALU.mult)
            with nc.allow_low_precision("bf16 stage-1 partial sums of 16"):
                nc.vector.tensor_reduce(
                    out=s1.rearrange("p t g -> p (t g)"), in_=sq, op=ALU.add,
                    axis=AX.X,
                )
            nc.vector.reduce_sum(out=ssq[:, qsl], in_=s1, axis=AX.X)

        sqscr = big.tile([128, D], f32)  # ACT Square scratch output

        def chunk_dve(dst, t0, ntiles, ssq0, act_sumsq=False):
            sl = slice(t0, t0 + ntiles)
            qsl = slice(ssq0, ssq0 + ntiles)
            if act_sumsq:
                # Fill-phase sumsq on the (idle) ACT engine: Square with
                # f32 accumulator, one instruction per 128x256 tile.
                for j in range(ntiles):
                    nc.scalar.activation(
                        out=sqscr, in_=dst[:, t0 + j, :], func=AF.Square,
                        accum_out=ssq[:, ssq0 + j : ssq0 + j + 1],
                    )
            else:
                two_stage_sumsq(dst[:, sl, :], ntiles, qsl, "sq")
            nc.scalar.activation(out=lnssq[:, qsl], in_=ssq[:, qsl], func=AF.Ln)
            nc.scalar.activation(
                out=invn[:, qsl], in_=lnssq[:, qsl], func=AF.Exp, scale=-0.5
            )
            z8 = f8p.tile([128, ntiles, D], fp8, tag="z8")
            for j in range(ntiles):
                nc.vector.tensor_scalar_mul(
                    out=z8[:, j, :], in0=dst[:, t0 + j, :],
                    scalar1=invn[:, ssq0 + j : ssq0 + j + 1],
                )
            return z8

        def chunk_pipeline(cc):
            """sumsq -> invn -> normalize-to-fp8 -> XBAR pair-transpose:
            repsT[p, 8cc+t, r] = pair (2p, 2p+1) of (tile 8cc+t, row r).
            Two half-chunk transposes so the first sim strip of the
            consuming step unlocks before the whole chunk lands."""
            z8 = chunk_dve(zf, cc * CH, CH, cc * CH)
            for hh in range(2):
                nc.sync.dma_start_transpose(
                    out=repsT[:, cc * CH + 4 * hh : cc * CH + 4 * (hh + 1), :],
                    in_=z8[:, 4 * hh : 4 * (hh + 1), :].bitcast(bf16),
                )

        def pe_transpose(z8, ntiles):
            """PE transpose of the fake-bf16 pair tiles via matmul x
            identity (bf16 -> f32 PSUM is exact); same pair layout as the
            XBAR after a cast-copy.  Used only before the main loop."""
            tps = pmm.tile([128, 1024], f32, tag="ps")
            z8b = z8[:, :, :].bitcast(bf16)
            for t in range(ntiles):
                nc.tensor.matmul(
                    out=tps[:, t * 128 : (t + 1) * 128],
                    lhsT=z8b[:, t, :], rhs=ident,
                    start=True, stop=True,
                )
            return tps

        def pair_ap(ap_bf16):
            """[128, t, 128] fake-bf16 -> [128(K), 2(slot), t*128] fp8 AP
            for DoubleRow matmul operands."""
            return ap_bf16.bitcast(fp8).rearrange("p t (r b) -> p b (t r)", b=2)

        # ---- first loads; later loads are emitted interleaved with the
        # chunk pipelines so each XBAR transpose's conservative cross-DMA
        # ordering dep lands on an already-completed load ----
        nc.gpsimd.dma_start(out=zbn[:, :, :], in_=zb_r)
        for cc in range(3):
            nc.gpsimd.dma_start(out=zf[:, cc * CH : (cc + 1) * CH, :], in_=chunk_src(cc))

        # ---- per-core row block first: it gates every main-loop matmul.
        # The stationary is RAW fp8 (no normalization): the row norm is
        # folded into the exp as a per-partition vector scale 2*invn_a,
        # so the PE-transpose chain starts right after the zb load instead
        # of after zb's sumsq.  PE transpose + direct PSUM repack into the
        # slot-major contiguous stationary (LDWEIGHTS dual-fp8 rejects
        # stride-2 operands). ----
        znbT8 = big.tile([128, 2, R], fp8)
        z8zb = chunk_dve(zbn, 0, TB, TF, act_sumsq=True)
        tzb = pe_transpose(z8zb, TB)

        # ---- chunks 0/1 bypass the XBAR (PE transpose + DVE cast-copy)
        # so the XBAR queue starts on chunk 2 and never gates the fill;
        # c1's PE work is deferred into main step 0 (it only gates step 1).
        # Chunk 0's pipeline runs before the znbT8 repack on DVE: both
        # gate the first sim matmul, but c0's chain is longer.
        def pe_chunk_finish(cc):
            tpsc = pe_transpose(_z8c[cc], CH)
            nc.vector.tensor_copy(
                out=repsT[:, cc * CH : (cc + 1) * CH, :],
                in_=tpsc[:, 0 : CH * 128],
            )

        _z8c = {}
        _z8c[0] = chunk_dve(zf, 0, CH, 0, act_sumsq=True)
        pe_chunk_finish(0)
        nc.vector.tensor_copy(
            out=znbT8,
            in_=tzb.bitcast(fp8).rearrange("p (x q) -> p q x", q=4)[
                :, 2:4, 0 : TB * 128
            ],
        )

        # Selector stationaries for the colsum matmuls: selq[:, m, j] is 1
        # only at j == m, so ones^T @ E lands in partition row m of the
        # long-lived [16, 512] PSUM accumulator (matmul output base
        # partition must be 0) while the other rows accumulate zeros.
        # Built here so the tiny DVE ops stay off the fill critical path.
        with nc.allow_low_precision("exact: rows of identity sum to 1.0"):
            nc.vector.reduce_sum(out=ones_b, in_=ident, axis=AX.X)
        nc.scalar.memzero(selq)
        nc.scalar.memzero(selp)
        for m in range(NCS):
            nc.vector.tensor_copy(out=selq[:, m, m : m + 1], in_=ones_b)
            for s in range(2):
                nc.vector.tensor_copy(out=selp[:, m, s, m : m + 1], in_=ones_b)

        # ---- main loop: column-outer over chunks; symmetric cover.
        # Emission order per step: sim matmuls + exps for C, with the
        # colsum matmuls of step C-1 interleaved between blocks (their E
        # tiles are long done, so they keep the PE streaming instead of
        # stalling on this step's exps), then chunk C+2's preprocessing
        # pipeline (so its LN/EXP sit behind this step's exps in the ACT
        # queue).  Pairs of E tiles are byte-interleaved so one DoubleRow
        # ones-matmul column-sums both at once. ----
        cs_first = True
        pend = []  # deferred colsum matmul thunks from the previous chunk

        def colsum_mm(Cp, h, entry, stop):
            nonlocal cs_first
            kind, e = entry
            m = 2 * Cp + h
            if kind == "pair":
                nc.tensor.matmul(
                    out=csacc[:, :],
                    lhsT=selp[:, m, :, :],
                    rhs=e[:, h * 512 : (h + 1) * 512, :].rearrange(
                        "p x b -> p b x"
                    ),
                    start=cs_first, stop=stop,
                    perf_mode=mybir.MatmulPerfMode.DoubleRow,
                    skip_group_check=True,
                )
            else:
                nc.tensor.matmul(
                    out=csacc[:, :],
                    lhsT=selq[:, m, :],
                    rhs=e[:, h * 512 : (h + 1) * 512],
                    start=cs_first, stop=stop,
                    skip_group_check=True,
                )
            cs_first = False

        for C in range(NCH):
            if C + 3 < NCH:
                nc.gpsimd.dma_start(
                    out=zf[:, (C + 3) * CH : (C + 4) * CH, :], in_=chunk_src(C + 3)
                )
            ks = [(C - d) % NCH for d in range(4)]
            if C >= 4:
                ks.append(C - 4)
            cj = [k for k in ks if k != C]
            # pair up the colsum jobs: (cj0,cj1) share one byte-interleaved
            # tile, (cj2,cj3) another; a leftover third job stays single.
            pair_of = {}
            for i in range(0, len(cj) - 1, 2):
                ep = epool.tile([128, 1024, 2], fp8, tag="ep")
                pair_of[cj[i]] = (ep, 0)
                pair_of[cj[i + 1]] = (ep, 1)
            jobs = []
            for bi, k in enumerate(ks):
                ps = pmm.tile([128, 1024], f32, tag="ps")
                for h in range(2):
                    nc.tensor.matmul(
                        out=ps[:, h * 512 : (h + 1) * 512],
                        lhsT=znbT8[:, :, k * 128 : (k + 1) * 128],
                        rhs=pair_ap(repsT[:, C * CH + 4 * h : C * CH + 4 * h + 4, :]),
                        start=True, stop=True,
                        perf_mode=mybir.MatmulPerfMode.DoubleRow,
                    )
                slot = (C - k) % NCH
                if k == C:
                    e_out = epool.tile([128, 1024], fp8, tag="e")
                    out_ap = e_out
                elif k in pair_of:
                    ep, s = pair_of[k]
                    out_ap = ep[:, :, s]
                    if s == 1:
                        jobs.append(("pair", ep))
                else:
                    e_out = epool.tile([128, 1024], fp8, tag="e")
                    out_ap = e_out
                    jobs.append(("single", e_out))
                # exp(2*sim) in [e^-4, e^4] sits inside fp8e4m3 range; the
                # per-element quantization noise is zero-mean and averages
                # out across the 512-term column sums.  The row norm
                # 2*invn_a rides in as a per-partition vector scale.
                nc.scalar.activation(
                    out=out_ap, in_=ps, func=AF.Exp, scale=2.0,
                    accum_out=Spart[:, k, slot : slot + 1],
                )
                if pend:
                    colsum_mm(*pend.pop(0), stop=False)
            if C == 0:
                _z8c[1] = chunk_dve(zf, CH, CH, CH, act_sumsq=True)
                pe_chunk_finish(1)
            while pend:
                colsum_mm(*pend.pop(0), stop=False)
            pend = [(C, h, entry) for h in range(2) for entry in jobs]
            if C + 2 < NCH:
                chunk_pipeline(C + 2)
        for i, (Cp, h, entry) in enumerate(pend):
            colsum_mm(Cp, h, entry, stop=(i == len(pend) - 1))

        # ---- self-diagonal exp(2*|zn|^2) from ssq * invn^2 (f32, [128,TB]) ----
        qz = slice(TF, TF + TB)
        d1 = big.tile([128, TB], f32)
        dacc = big.tile([128, TB], f32)
        nc.vector.tensor_mul(out=d1, in0=ssq[:, qz], in1=invn[:, qz])
        nc.vector.tensor_mul(out=dacc, in0=d1, in1=invn[:, qz])
        expd = big.tile([128, TB], f32)
        nc.scalar.activation(out=expd, in_=dacc, func=AF.Exp, scale=2.0)

        # ---- positive pairs: raw dots * invn_k * invn_{k+4}, local pairs ----
        posp = sqp.tile([128, TH, D // RED1, RED1], bf16, tag="sq")
        pos1 = sqp.tile([128, TH, D // RED1], bf16, tag="sq1")
        posr = big.tile([128, TH], f32)
        zl = zbn[:, 0:TH, :].rearrange("p t (g r) -> p t g r", r=RED1)
        zh = zbn[:, TH:TB, :].rearrange("p t (g r) -> p t g r", r=RED1)
        nc.vector.tensor_tensor(out=posp, in0=zl, in1=zh, op=ALU.mult)
        with nc.allow_low_precision("bf16 stage-1 partial sums of 16"):
            nc.vector.tensor_reduce(out=pos1, in_=posp, op=ALU.add, axis=AX.X)
        nc.vector.reduce_sum(out=posr, in_=pos1, axis=AX.X)
        ps1 = big.tile([128, TH], f32)
        ps2 = big.tile([128, TH], f32)
        posd = big.tile([128, TH], f32)
        nc.vector.tensor_mul(out=ps1, in0=posr, in1=invn[:, TF : TF + TH])
        nc.vector.tensor_mul(out=ps2, in0=ps1, in1=invn[:, TF + TH : TF + TB])
        nc.vector.tensor_scalar_mul(out=posd, in0=ps2, scalar1=2.0)
        nc.sync.dma_start(out=pos_out[:, :], in_=posd)

        nc.vector.tensor_copy(out=cs_sb, in_=csacc[:, :])
        nc.sync.dma_start(out=cs_out[:, :], in_=cs_sb)

        # ---- S' = sum - diag, ship out ----
        S_t = big.tile([128, TB], f32)
        nc.vector.reduce_sum(out=S_t, in_=Spart[:, :, :], axis=AX.X)
        lse_in_t = big.tile([128, TB], f32)
        nc.vector.tensor_sub(out=lse_in_t, in0=S_t, in1=expd)
        nc.sync.dma_start(out=lse_out[:, :], in_=lse_in_t)

    return nc


_NC_CACHE = {}


def _get_nc(N=N_FULL, D=D_FULL):
    key = (N, D)
    if key not in _NC_CACHE:
        _NC_CACHE[key] = build_bass(N, D)
    return _NC_CACHE[key]


def make_in_maps(z_i, z_j, n_cores=N_CORES):
    z_i = np.ascontiguousarray(z_i, dtype=np.float32)
    z_j = np.ascontiguousarray(z_j, dtype=np.float32)
    reps = np.concatenate([z_i, z_j], axis=0)
    TB = reps.shape[0] // 128 // n_cores
    maps = []
    for c in range(n_cores):
        # core c owns global tiles {c + 8k}; tile g holds rows
        # sigma(g, p) = (g // 8) * 1024 + p * 8 + (g % 8).  zb row
        # (p*TB + k) feeds (partition p, local tile k).
        idx = np.empty(128 * TB, dtype=np.int64)
        for p in range(128):
            for k in range(TB):
                idx[p * TB + k] = k * 1024 + p * 8 + c
        maps.append({"z_i": z_i, "z_j": z_j, "zb": np.ascontiguousarray(reps[idx])})
    return maps


def assemble(results, N=N_FULL, n_cores=N_CORES):
    """Host-side gather + final ln/mean ("all-reduce the mean loss")."""
    n2 = 2 * N
    TB = n2 // 128 // n_cores
    lse = np.empty(n2, dtype=np.float64)
    pos = np.empty(n2, dtype=np.float64)
    colsum = np.zeros(n2, dtype=np.float64)
    p_ar = np.arange(128)
    for c, r in enumerate(results):
        lse_in = np.asarray(r["lse_in"], dtype=np.float64)   # [128, TB]
        posd = np.asarray(r["posd"], dtype=np.float64)       # [128, TB//2]
        for k in range(TB):
            rows = k * 1024 + p_ar * 8 + c   # sigma(c + 8k, p)
            lse[rows] = lse_in[:, k]
            pos[rows] = posd[:, k % (TB // 2)]
        # colsum[2C+h, q]: within-chunk col j = h*512 + q = tc*128 + rr
        # -> global row C*1024 + rr*8 + tc
        cs = np.asarray(r["colsum"], dtype=np.float64).reshape(8, 2, 4, 128)
        colsum += np.transpose(cs, (0, 3, 1, 2)).reshape(n2)
    loss = np.mean(np.log(lse + colsum) - pos)
    return np.float32(loss)


def _run(z_i, z_j, trace=False, tmpdir=None, **spmd_kwargs):
    from concourse.bass_utils import run_bass_kernel_spmd

    N, D = z_i.shape
    nc = _get_nc(N, D)
    in_maps = make_in_maps(z_i, z_j)
    out = run_bass_kernel_spmd(
        nc, in_maps, list(range(N_CORES)), trace=trace, tmpdir=tmpdir, **spmd_kwargs
    )
    return assemble(out.results, N), out


def kernel(z_i, z_j):
    loss, _ = _run(np.asarray(z_i), np.asarray(z_j))
    return loss


if __name__ == "__main__":
    rng = np.random.default_rng(0)
    z_i = rng.standard_normal((N_FULL, D_FULL), dtype=np.float32)
    z_j = rng.standard_normal((N_FULL, D_FULL), dtype=np.float32)
    print(kernel(z_i, z_j))
